# revision 1
# baseline (speedup 1.0000x reference)
"""Cross-attention kernel for 8 Trainium2 NeuronCores.

Reference computation (per batch element a):
  K = X @ Wk, Q = L @ Wq, V = X @ Wv          (each head uses a full 256-dim slice)
  S_i = Q_i @ K_i^T / sqrt(32); P = softmax(S); A_i = P_i @ V_i
  out = concat_i(A_i) @ Wu + bu

Sharding: core c = 2*a + hg handles batch a and head-group hg (4 heads, 1024
projection columns). The final head-concat matmul is split over head-groups;
the two partial outputs per batch element are summed on the host (the
"all-reduce after unify_heads"), which also adds the bias.

All matmuls run as float32r (fp32 storage; PE processes 1 row/cycle at
N>=256 vs 4 cycles/row for plain fp32, with identical numerics on TRN2 --
both use the same reduced-precision PE path, measured ~1.6e-4 max rel err).

Device layouts (per core) -- everything keeps the contraction dim on SBUF
partitions so no on-device transposes are needed:
  XT, LT           [256, 1024]  x^T / latent^T   (host pre-transposes)
  WK, WQ, WV       [256, 1024]  natural
  WU               [1024, 256]  natural
  KT = (X@WK)^T    [1024(n), 1024(s)]  via lhsT=WK-tile, rhs=XT
  QT = (L@WQ)^T    [1024(n), 1024(y)]  via lhsT=WQ-tile, rhs=LT
  V  = X@WV        [1024(s), 1024(n)]  via lhsT=XT-tile, rhs=WV
  S^T_i            [b, y] psum via lhsT=KT-tile, rhs=QT        (per head i)
  P^T_i = exp(.)   [b, y] sbuf, ACT exp with scale fused
  sums_i           [1, y] via lhsT=ones column  (softmax denominators)
  A^T_i            [c, y] psum via lhsT=V-tile, rhs=P^T; normalized by
                   1/sums (broadcast via K=1 ones matmul) on PSUM->SBUF copy
  O               [y, e] via lhsT=A^T-tile, rhs=WU-tile
"""

import math
import sys

import numpy as np

sys.path.insert(0, "/opt/trn_rl_repo")

import concourse.bass as bass  # noqa: E402
import concourse.mybir as mybir  # noqa: E402
from concourse import bacc, bass_isa  # noqa: E402
from concourse.bass_utils import run_bass_kernel_spmd  # noqa: E402
from concourse.tile import TileContext  # noqa: E402

F32 = mybir.dt.float32
F32R = mybir.dt.float32r
EXP = mybir.ActivationFunctionType.Exp

B, S, E = 4, 1024, 256          # batch, seq, embed
HEADS = 8                        # total heads; each head dim = E (source quirk)
N_CORES = 8
HG_HEADS = 4                     # heads per head-group (per core)
NH = HG_HEADS * E                # projection columns per core = 1024
SCALE = 1.0 / math.sqrt(E // HEADS)   # 1/sqrt(32)

P = 128                          # SBUF partitions
NT = NH // P                     # 8 partition tiles of the projection dim
ST = S // P                      # 8 partition tiles of the seq dim
NCH = 512                        # matmul moving-dim chunk
SCH = S // NCH                   # 2 chunks of 512 over seq

_CACHE = {}

import os as _os
SCRATCH_BUFS = int(_os.environ.get("K_SCRATCH", "10"))
SC_BUFS = int(_os.environ.get("K_SC", "4"))
PA_BUFS = int(_os.environ.get("K_PA", "3"))
MISC_BUFS = int(_os.environ.get("K_MISC", "2"))


def _build():
    nc = bacc.Bacc(target_bir_lowering=False)

    XT = nc.dram_tensor("XT", [E, S], F32R, kind="ExternalInput")
    LT = nc.dram_tensor("LT", [E, S], F32R, kind="ExternalInput")
    WK = nc.dram_tensor("WK", [E, NH], F32R, kind="ExternalInput")
    WQ = nc.dram_tensor("WQ", [E, NH], F32R, kind="ExternalInput")
    WV = nc.dram_tensor("WV", [E, NH], F32R, kind="ExternalInput")
    WU = nc.dram_tensor("WU", [NH, E], F32R, kind="ExternalInput")
    O = nc.dram_tensor("O", [S, E], F32, kind="ExternalOutput")

    ET = E // P  # 2 partition tiles of the embed (contraction) dim

    with TileContext(nc) as tc:
        with tc.tile_pool(name="persist", bufs=1) as pp, \
             tc.tile_pool(name="scratch", bufs=SCRATCH_BUFS) as sp, \
             tc.tile_pool(name="small", bufs=2) as mp, \
             tc.tile_pool(name="psum", bufs=1, space="PSUM") as ps:

            # ---- phase 0: load inputs (chunked so compute starts early) ----
            def alloc_in(nm):
                return [sp.tile([P, S], F32R, tag="big", name=f"{nm}{t}")
                        for t in range(ET)]

            xt, wk, wv, lt, wq = (alloc_in(n) for n in ("xt", "wk", "wv", "lt", "wq"))

            def dma_chunk(tiles, dram, e, c):
                nc.sync.dma_start(
                    out=tiles[e][:, c * NCH:(c + 1) * NCH],
                    in_=dram[e * P:(e + 1) * P, c * NCH:(c + 1) * NCH])

            # issue order: operands of the first KT groups first
            for e in range(ET):
                nc.sync.dma_start(out=wk[e][:, 0:P],
                                  in_=WK[e * P:(e + 1) * P, 0:P])
            for e in range(ET):
                dma_chunk(xt, XT, e, 0)
            for e in range(ET):
                nc.sync.dma_start(out=wk[e][:, P:NCH],
                                  in_=WK[e * P:(e + 1) * P, P:NCH])
            for e in range(ET):
                dma_chunk(wk, WK, e, 1)
            for e in range(ET):
                dma_chunk(xt, XT, e, 1)
            # lt/wq/wv go through the SWDGE (gpsimd) queue: its descriptor
            # generation runs in parallel with the HWDGE queue handling the
            # fill-critical xt/wk stream
            for c in range(SCH):
                for e in range(ET):
                    dma_chunk(wv, WV, e, c)
            for c in range(SCH):
                for e in range(ET):
                    dma_chunk(lt, LT, e, c)
            for c in range(SCH):
                for e in range(ET):
                    dma_chunk(wq, WQ, e, c)
            wu_all = pp.tile([P, NT * E], F32R, tag="wu", name="wu_all")
            nc.sync.dma_start(out=wu_all[:].rearrange("p (t e) -> p t e", t=NT),
                              in_=WU.rearrange("(t p) e -> p t e", p=P))
            wu = [wu_all[:, t * E:(t + 1) * E] for t in range(NT)]

            # ---- phase 1: projections KT, QT (transposed), V (natural) ----
            kt, qt, v = [], [], []
            for nt in range(NT):
                kt.append(pp.tile([P, S], F32R, tag=f"kt{nt}", name=f"kt{nt}"))
                qt.append(pp.tile([P, S], F32R, tag=f"qt{nt}", name=f"qt{nt}"))
                v.append(pp.tile([P, S], F32R, tag=f"v{nt}", name=f"v{nt}"))

            # alternate PSUM->SBUF evictions between DVE and ACT so neither
            # engine gates the PE during the projection phase
            evict_ctr = [0]

            def evict(dst_ap, src_ap):
                evict_ctr[0] += 1
                if evict_ctr[0] % 2 == 0:
                    nc.vector.tensor_copy(dst_ap, src_ap)
                else:
                    nc.scalar.activation(dst_ap, src_ap,
                                         mybir.ActivationFunctionType.Copy)

            def proj(dst, lhs_tiles, lhs_cols, rhs_tiles, nt, c, nm):
                sl = bass.ts(c, NCH)
                pk = ps.tile([P, NCH], F32, tag="pA", bufs=PA_BUFS, name=f"p{nm}{nt}{c}")
                for e in range(ET):
                    nc.tensor.matmul(pk[:], lhs_tiles[e][:, lhs_cols],
                                     rhs_tiles[e][:, sl],
                                     start=(e == 0), stop=(e == ET - 1))
                evict(dst[nt][:, sl], pk[:])

            for c in range(SCH):
                for nt in range(NT):
                    proj(kt, wk, slice(nt * P, (nt + 1) * P), xt, nt, c, "k")
            for c in range(SCH):
                for nt in range(NT):
                    proj(v, xt, slice(nt * P, (nt + 1) * P), wv, nt, c, "v")
            for c in range(SCH):
                for nt in range(NT):
                    proj(qt, wq, slice(nt * P, (nt + 1) * P), lt, nt, c, "q")

            # ---- phase 2: attention per head ----
            at = []
            for nt in range(NT):
                at.append(pp.tile([P, S], F32R, tag=f"at{nt}", name=f"at{nt}"))

            o_acc = []

            pt_h = {}
            acc_h = {}
            acc2_h = {}
            sums_h = {}
            rec_h = {}

            def st_group(h, c, bt):
                c0 = 2 * h
                sl = bass.ts(c, NCH)
                pt, acc = pt_h[h], acc_h[h]
                pss = ps.tile([P, NCH], F32, tag="sc", bufs=SC_BUFS,
                              name=f"pss{h}{bt}{c}")
                for cj in range(2):
                    nc.tensor.matmul(pss[:], kt[c0 + cj][:, bt * P:(bt + 1) * P],
                                     qt[c0 + cj][:, sl],
                                     start=(cj == 0), stop=(cj == 1))
                nc.scalar.activation(pt[bt][:, sl], pss[:], EXP, scale=SCALE)
                # denominator accumulation pipelined with the exps; the
                # two half-trees run on DVE and the (otherwise idle) GPSIMD
                acc2 = acc2_h[h]
                if bt == 1:
                    nc.vector.tensor_add(acc[:, sl], pt[0][:, sl], pt[1][:, sl])
                elif bt in (2, 3):
                    nc.vector.tensor_add(acc[:, sl], acc[:, sl], pt[bt][:, sl])
                elif bt == 5:
                    nc.gpsimd.tensor_add(acc2[:, sl], pt[4][:, sl], pt[5][:, sl])
                elif bt in (6, 7):
                    nc.gpsimd.tensor_add(acc2[:, sl], acc2[:, sl], pt[bt][:, sl])

            def sums_chain(h, c):
                # softmax denominators: single GPSIMD all-reduce over the
                # partition dim (broadcasting the sum to all partitions),
                # then invert on DVE
                sl = bass.ts(c, NCH)
                nc.vector.tensor_add(acc_h[h][:, sl], acc_h[h][:, sl],
                                     acc2_h[h][:, sl])
                nc.gpsimd.partition_all_reduce(
                    rec_h[h][:, sl], acc_h[h][:, sl].bitcast(F32),
                    channels=P, reduce_op=bass_isa.ReduceOp.add)
                nc.vector.reciprocal(rec_h[h][:, sl], rec_h[h][:, sl])

            def at_group(h, c, ct):
                # A^T accumulation over b; normalize on PSUM->SBUF eviction
                sl = bass.ts(c, NCH)
                vsl = slice(h * E + ct * P, h * E + (ct + 1) * P)
                pa = ps.tile([P, NCH], F32, tag="pA", bufs=PA_BUFS,
                             name=f"pa{h}{ct}{c}")
                for bt in range(ST):
                    nc.tensor.matmul(pa[:], v[bt][:, vsl], pt_h[h][bt][:, sl],
                                     start=(bt == 0), stop=(bt == ST - 1))
                nc.vector.tensor_mul(at[2 * h + ct][:, sl], pa[:],
                                     rec_h[h][:, sl])

            def head_alloc(h):
                pt_h[h] = [sp.tile([P, S], F32R, tag="big", name=f"pt{h}{bt}")
                           for bt in range(ST)]
                acc_h[h] = mp.tile([P, S], F32R, tag="sacc", name=f"sacc{h}")
                acc2_h[h] = mp.tile([P, S], F32R, tag="sacc2", name=f"sacc2{h}")
                rec_h[h] = mp.tile([P, S], F32, tag="rec", bufs=2,
                                   name=f"rec{h}")

            def out_tile(yt):
                po = ps.tile([P, E], F32, tag="sc", bufs=SC_BUFS,
                             name=f"po{yt}")
                for ht in range(NT):
                    nc.tensor.matmul(po[:], at[ht][:, yt * P:(yt + 1) * P],
                                     wu[ht],
                                     start=(ht == 0), stop=(ht == NT - 1))
                osb = mp.tile([P, E], F32, tag="osb", bufs=6, name=f"osb{yt}")
                evict(osb[:], po[:])
                nc.sync.dma_start(out=O[yt * P:(yt + 1) * P, :], in_=osb[:])

            # cross-head software pipeline: head h's chunk-1 A^T groups are
            # woven into head h+1's chunk-0 score stream, so the PE always has
            # matmul work while ACT drains the exp queue
            for h in range(HG_HEADS):
                head_alloc(h)
                for bt in range(ST):
                    st_group(h, 0, bt)
                    if h > 0:
                        if bt == 2:
                            at_group(h - 1, 1, 0)
                        elif bt == 5:
                            at_group(h - 1, 1, 1)
                sums_chain(h, 0)
                for bt in range(ST):
                    st_group(h, 1, bt)
                    if bt == 2:
                        at_group(h, 0, 0)
                    elif bt == 5:
                        at_group(h, 0, 1)
                sums_chain(h, 1)
            LAST = HG_HEADS - 1

            # ---- phase 3: output tiles woven with head-3's final A^T ----
            out_tile(0)
            out_tile(1)
            at_group(LAST, 1, 0)
            out_tile(2)
            out_tile(3)
            at_group(LAST, 1, 1)
            for yt in range(4, ST):
                out_tile(yt)

    nc.compile()
    return nc


def kernel(batch, latent, Wk, Wq, Wv, Wu, bu):
    batch = np.asarray(batch, dtype=np.float32)
    latent = np.asarray(latent, dtype=np.float32)
    Wk = np.asarray(Wk, dtype=np.float32)
    Wq = np.asarray(Wq, dtype=np.float32)
    Wv = np.asarray(Wv, dtype=np.float32)
    Wu = np.asarray(Wu, dtype=np.float32)
    bu = np.asarray(bu, dtype=np.float32)

    if "nc" not in _CACHE:
        _CACHE["nc"] = _build()
    nc = _CACHE["nc"]

    in_maps = []
    for core in range(N_CORES):
        a, hg = core // 2, core % 2
        cols = slice(hg * NH, (hg + 1) * NH)
        in_maps.append({
            "XT": np.ascontiguousarray(batch[a].T),
            "LT": np.ascontiguousarray(latent[a].T),
            "WK": np.ascontiguousarray(Wk[:, cols]),
            "WQ": np.ascontiguousarray(Wq[:, cols]),
            "WV": np.ascontiguousarray(Wv[:, cols]),
            "WU": np.ascontiguousarray(Wu[cols, :]),
        })

    _CACHE["in_maps"] = in_maps
    res = run_bass_kernel_spmd(nc, in_maps, core_ids=list(range(N_CORES)))

    out = np.empty((B, S, E), dtype=np.float32)
    for a in range(B):
        out[a] = res.results[2 * a]["O"] + res.results[2 * a + 1]["O"] + bu
    return out



# revision 3
# speedup vs baseline: 1.8248x; 1.8248x over previous
"""Cross-attention kernel for 8 Trainium2 NeuronCores.

Reference computation (per batch element a):
  K = X @ Wk, Q = L @ Wq, V = X @ Wv          (each head uses a full 256-dim slice)
  S_i = Q_i @ K_i^T / sqrt(32); P = softmax(S); A_i = P_i @ V_i
  out = concat_i(A_i) @ Wu + bu
Sharding: core c = 2*a + hg handles batch a and head-group hg (4 heads, 1024
projection columns). The two partial outputs per batch element are summed on
the host (the "all-reduce after unify_heads"), which also adds the bias.

All operands are bf16 (PE runs 1 row/cycle, same as fp32r, but weight loads,
DMA and eviction bytes halve; measured end-to-end rel err ~4e-3 vs the 2e-2
gate). PSUM stays fp32.

Softmax denominators run on the PE: an all-ones [128,128] stationary tile
accumulates column sums of each exp'd P^T tile into a PSUM bank (producing the
sum broadcast across all 128 partitions directly), woven into the score
matmul stream.  A fast DVE reciprocal then yields 1/sum, and the PV-matmul
eviction multiplies it in on GPSIMD.  This removes the partition_all_reduce /
full-tile reciprocal / add-tree chain that serialized each head-chunk.

Device layouts (per core) -- contraction dim always on SBUF partitions:
  XT, LT           [256, 1024]  x^T / latent^T   (host pre-transposes)
  WK, WQ, WV       [256, 1024], WU [1024, 256]   natural
  KT = (X@WK)^T    [1024(n), 1024(s)] ; QT likewise ; V = X@WV [1024(s),1024(n)]
  S^T_i            [b, y] psum;  P^T_i = exp(.)  [b, y] sbuf bf16
  sums_i           [128, y] psum via ones[128,128] lhsT (broadcast col-sums)
  A^T_i            [c, y] psum; normalized by 1/sums on PSUM->SBUF eviction
  O                [y, e] fp32
"""

import math
import sys

import numpy as np

sys.path.insert(0, "/opt/trn_rl_repo")

import ml_dtypes  # noqa: E402

import concourse.bass as bass  # noqa: E402
import concourse.mybir as mybir  # noqa: E402
from concourse import bacc  # noqa: E402
from concourse.bass_utils import run_bass_kernel_spmd  # noqa: E402
from concourse.tile import TileContext  # noqa: E402

F32 = mybir.dt.float32
BF16 = mybir.dt.bfloat16
EXP = mybir.ActivationFunctionType.Exp
COPY = mybir.ActivationFunctionType.Copy

B, S, E = 4, 1024, 256          # batch, seq, embed
HEADS = 8                        # total heads; each head dim = E (source quirk)
N_CORES = 8
HG_HEADS = 4                     # heads per head-group (per core)
NH = HG_HEADS * E                # projection columns per core = 1024
SCALE = 1.0 / math.sqrt(E // HEADS)   # 1/sqrt(32)

P = 128                          # SBUF partitions
NT = NH // P                     # 8 partition tiles of the projection dim
ST = S // P                      # 8 partition tiles of the seq dim
NCH = 512                        # matmul moving-dim chunk
SCH = S // NCH                   # 2 chunks of 512 over seq

_CACHE = {}

import os as _os
SCRATCH_BUFS = int(_os.environ.get("K_SCRATCH", "12"))
SC_BUFS = int(_os.environ.get("K_SC", "3"))
PA_BUFS = int(_os.environ.get("K_PA", "3"))
SR_BUFS = int(_os.environ.get("K_SR", "2"))


def _build():
    nc = bacc.Bacc(target_bir_lowering=False)

    XT = nc.dram_tensor("XT", [E, S], BF16, kind="ExternalInput")
    LT = nc.dram_tensor("LT", [E, S], BF16, kind="ExternalInput")
    WK = nc.dram_tensor("WK", [E, NH], BF16, kind="ExternalInput")
    WQ = nc.dram_tensor("WQ", [E, NH], BF16, kind="ExternalInput")
    WV = nc.dram_tensor("WV", [E, NH], BF16, kind="ExternalInput")
    WU = nc.dram_tensor("WU", [NH, E], BF16, kind="ExternalInput")
    O = nc.dram_tensor("O", [S, E], F32, kind="ExternalOutput")

    ET = E // P  # 2 partition tiles of the embed (contraction) dim

    with TileContext(nc) as tc:
        with tc.tile_pool(name="persist", bufs=1) as pp, \
             tc.tile_pool(name="scratch", bufs=SCRATCH_BUFS) as sp, \
             tc.tile_pool(name="small", bufs=2) as mp, \
             tc.tile_pool(name="psum", bufs=1, space="PSUM") as ps:

            ones = pp.tile([P, P], BF16, tag="ones", name="ones")
            nc.gpsimd.memset(ones[:], 1.0)

            # ---- phase 0: load inputs (chunked so compute starts early) ----
            def alloc_in(nm):
                return [sp.tile([P, S], BF16, tag="big", name=f"{nm}{t}")
                        for t in range(ET)]

            xt, wk, wv, lt, wq = (alloc_in(n) for n in ("xt", "wk", "wv", "lt", "wq"))

            def dma_chunk(tiles, dram, e, c):
                nc.sync.dma_start(
                    out=tiles[e][:, c * NCH:(c + 1) * NCH],
                    in_=dram[e * P:(e + 1) * P, c * NCH:(c + 1) * NCH])

            # issue order: operands of the first KT groups first
            for e in range(ET):
                nc.sync.dma_start(out=wk[e][:, 0:P],
                                  in_=WK[e * P:(e + 1) * P, 0:P])
            for e in range(ET):
                dma_chunk(xt, XT, e, 0)
            for e in range(ET):
                nc.sync.dma_start(out=wk[e][:, P:NCH],
                                  in_=WK[e * P:(e + 1) * P, P:NCH])
            for e in range(ET):
                dma_chunk(wk, WK, e, 1)
            for e in range(ET):
                dma_chunk(xt, XT, e, 1)
            for c in range(SCH):
                for e in range(ET):
                    dma_chunk(wv, WV, e, c)
            for c in range(SCH):
                for e in range(ET):
                    dma_chunk(lt, LT, e, c)
            for c in range(SCH):
                for e in range(ET):
                    dma_chunk(wq, WQ, e, c)
            wu_all = pp.tile([P, NT * E], BF16, tag="wu", name="wu_all")
            nc.sync.dma_start(out=wu_all[:].rearrange("p (t e) -> p t e", t=NT),
                              in_=WU.rearrange("(t p) e -> p t e", p=P))
            wu = [wu_all[:, t * E:(t + 1) * E] for t in range(NT)]

            # ---- phase 1: projections KT, QT (transposed), V (natural) ----
            kt, qt, v = [], [], []
            for nt in range(NT):
                kt.append(pp.tile([P, S], BF16, tag=f"kt{nt}", name=f"kt{nt}"))
                qt.append(pp.tile([P, S], BF16, tag=f"qt{nt}", name=f"qt{nt}"))
                v.append(pp.tile([P, S], BF16, tag=f"v{nt}", name=f"v{nt}"))

            # alternate PSUM->SBUF evictions between DVE and ACT so neither
            # engine gates the PE during the projection phase
            evict_ctr = [0]

            def evict(dst_ap, src_ap):
                evict_ctr[0] += 1
                if evict_ctr[0] % 2 == 0:
                    nc.vector.tensor_copy(dst_ap, src_ap)
                else:
                    nc.scalar.activation(dst_ap, src_ap, COPY)

            def proj(dst, lhs_tiles, lhs_cols, rhs_tiles, nt, c, nm):
                sl = bass.ts(c, NCH)
                pk = ps.tile([P, NCH], F32, tag="pa", bufs=PA_BUFS, name=f"p{nm}{nt}{c}")
                for e in range(ET):
                    nc.tensor.matmul(pk[:], lhs_tiles[e][:, lhs_cols],
                                     rhs_tiles[e][:, sl],
                                     start=(e == 0), stop=(e == ET - 1))
                evict(dst[nt][:, sl], pk[:])

            for c in range(SCH):
                for nt in range(NT):
                    proj(kt, wk, slice(nt * P, (nt + 1) * P), xt, nt, c, "k")
            for c in range(SCH):
                for nt in range(NT):
                    proj(v, xt, slice(nt * P, (nt + 1) * P), wv, nt, c, "v")
            for c in range(SCH):
                for nt in range(NT):
                    proj(qt, wq, slice(nt * P, (nt + 1) * P), lt, nt, c, "q")

            # ---- phase 2: attention per head ----
            at = []
            for nt in range(NT):
                at.append(pp.tile([P, S], BF16, tag=f"at{nt}", name=f"at{nt}"))

            pt_h = {}
            rec_h = {}
            srec = {}

            def st_group(h, c, bt):
                # scores for one b-tile + exp; weave the PE column-sum of the
                # previous b-tile's P^T into the stream
                c0 = 2 * h
                sl = bass.ts(c, NCH)
                pt = pt_h[h]
                pss = ps.tile([P, NCH], F32, tag="sc", bufs=SC_BUFS,
                              name=f"pss{h}{bt}{c}")
                for cj in range(2):
                    nc.tensor.matmul(pss[:], kt[c0 + cj][:, bt * P:(bt + 1) * P],
                                     qt[c0 + cj][:, sl],
                                     start=(cj == 0), stop=(cj == 1))
                nc.scalar.activation(pt[bt][:, sl], pss[:], EXP, scale=SCALE)
                if bt >= 1:
                    nc.tensor.matmul(srec[(h, c)][:], ones[:],
                                     pt[bt - 1][:, sl],
                                     start=(bt == 1), stop=False)

            def finish_sums(h, c):
                # last column-sum + fast reciprocal of the broadcast sums
                sl = bass.ts(c, NCH)
                nc.tensor.matmul(srec[(h, c)][:], ones[:], pt_h[h][7][:, sl],
                                 start=False, stop=True)
                nc.vector.reciprocal_approx_fast(rec_h[(h, c)][:],
                                                 srec[(h, c)][:])

            def at_group(h, c, ct):
                # A^T accumulation over b; normalize on PSUM->SBUF eviction
                sl = bass.ts(c, NCH)
                vsl = slice(h * E + ct * P, h * E + (ct + 1) * P)
                pa = ps.tile([P, NCH], F32, tag="pa", bufs=PA_BUFS,
                             name=f"pa{h}{ct}{c}")
                for bt in range(ST):
                    nc.tensor.matmul(pa[:], v[bt][:, vsl], pt_h[h][bt][:, sl],
                                     start=(bt == 0), stop=(bt == ST - 1))
                nc.vector.tensor_mul(at[2 * h + ct][:, sl], pa[:],
                                     rec_h[(h, c)][:])

            def head_alloc(h):
                pt_h[h] = [sp.tile([P, S], BF16, tag="big", name=f"pt{h}{bt}")
                           for bt in range(ST)]
                for c in range(SCH):
                    rec_h[(h, c)] = mp.tile([P, NCH], F32, tag="rec", bufs=3,
                                            name=f"rec{h}{c}")
                    srec[(h, c)] = ps.tile([P, NCH], F32, tag="sr",
                                           bufs=SR_BUFS, name=f"sr{h}{c}")

            def out_tile(yt):
                po = ps.tile([P, E], F32, tag="sc", bufs=SC_BUFS,
                             name=f"po{yt}")
                for ht in range(NT):
                    nc.tensor.matmul(po[:], at[ht][:, yt * P:(yt + 1) * P],
                                     wu[ht],
                                     start=(ht == 0), stop=(ht == NT - 1))
                osb = mp.tile([P, E], F32, tag="osb", bufs=6, name=f"osb{yt}")
                evict(osb[:], po[:])
                nc.sync.dma_start(out=O[yt * P:(yt + 1) * P, :], in_=osb[:])

            # cross-head software pipeline: head h's chunk-1 A^T groups are
            # woven into head h+1's chunk-0 score stream, so the PE always has
            # matmul work while ACT drains the exp queue
            for h in range(HG_HEADS):
                head_alloc(h)
                for bt in range(ST):
                    st_group(h, 0, bt)
                    if h > 0:
                        if bt == 2:
                            at_group(h - 1, 1, 0)
                        elif bt == 5:
                            at_group(h - 1, 1, 1)
                finish_sums(h, 0)
                for bt in range(ST):
                    st_group(h, 1, bt)
                    if bt == 2:
                        at_group(h, 0, 0)
                    elif bt == 5:
                        at_group(h, 0, 1)
                finish_sums(h, 1)
            LAST = HG_HEADS - 1

            # ---- phase 3: output tiles woven with head-3's final A^T ----
            out_tile(0)
            out_tile(1)
            at_group(LAST, 1, 0)
            out_tile(2)
            out_tile(3)
            at_group(LAST, 1, 1)
            for yt in range(4, ST):
                out_tile(yt)

    nc.compile()
    return nc


def kernel(batch, latent, Wk, Wq, Wv, Wu, bu):
    bf16 = ml_dtypes.bfloat16
    batch = np.asarray(batch, dtype=np.float32)
    latent = np.asarray(latent, dtype=np.float32)
    bu = np.asarray(bu, dtype=np.float32)

    if "nc" not in _CACHE:
        _CACHE["nc"] = _build()
    nc = _CACHE["nc"]

    xts = [np.ascontiguousarray(batch[a].T).astype(bf16) for a in range(B)]
    lts = [np.ascontiguousarray(latent[a].T).astype(bf16) for a in range(B)]
    wks, wqs, wvs, wus = [], [], [], []
    for hg in range(2):
        cols = slice(hg * NH, (hg + 1) * NH)
        wks.append(np.ascontiguousarray(np.asarray(Wk, np.float32)[:, cols]).astype(bf16))
        wqs.append(np.ascontiguousarray(np.asarray(Wq, np.float32)[:, cols]).astype(bf16))
        wvs.append(np.ascontiguousarray(np.asarray(Wv, np.float32)[:, cols]).astype(bf16))
        wus.append(np.ascontiguousarray(np.asarray(Wu, np.float32)[cols, :]).astype(bf16))

    in_maps = []
    for core in range(N_CORES):
        a, hg = core // 2, core % 2
        in_maps.append({
            "XT": xts[a], "LT": lts[a],
            "WK": wks[hg], "WQ": wqs[hg], "WV": wvs[hg], "WU": wus[hg],
        })

    _CACHE["in_maps"] = in_maps
    res = run_bass_kernel_spmd(nc, in_maps, core_ids=list(range(N_CORES)))

    out = np.empty((B, S, E), dtype=np.float32)
    for a in range(B):
        out[a] = res.results[2 * a]["O"] + res.results[2 * a + 1]["O"] + bu
    return out


# revision 8
# speedup vs baseline: 1.9987x; 1.0953x over previous
"""Cross-attention kernel for 8 Trainium2 NeuronCores.

Reference computation (per batch element a):
  K = X @ Wk, Q = L @ Wq, V = X @ Wv          (each head uses a full 256-dim slice)
  S_i = Q_i @ K_i^T / sqrt(32); P = softmax(S); A_i = P_i @ V_i
  out = concat_i(A_i) @ Wu + bu
Sharding: core c = 2*a + hg handles batch a and head-group hg (4 heads, 1024
projection columns). The two partial outputs per batch element are summed on
the host (the "all-reduce after unify_heads"), which also adds the bias.

All operands are bf16 (PE runs 1 row/cycle, same as fp32r, but weight loads,
DMA and eviction bytes halve; measured end-to-end rel err ~4e-3 vs the 2e-2
gate). PSUM stays fp32.

Softmax denominators run on the PE: an all-ones [128,128] stationary tile
accumulates column sums of each exp'd P^T tile into a PSUM bank (producing the
sum broadcast across all 128 partitions directly), woven into the score
matmul stream.  A fast DVE reciprocal then yields 1/sum, and the PV-matmul
eviction multiplies it in on GPSIMD.  This removes the partition_all_reduce /
full-tile reciprocal / add-tree chain that serialized each head-chunk.

Device layouts (per core) -- contraction dim always on SBUF partitions:
  XT, LT           [256, 1024]  x^T / latent^T   (host pre-transposes)
  WK, WQ, WV       [256, 1024], WU [1024, 256]   natural
  KT = (X@WK)^T    [1024(n), 1024(s)] ; QT likewise ; V = X@WV [1024(s),1024(n)]
  S^T_i            [b, y] psum;  P^T_i = exp(.)  [b, y] sbuf bf16
  sums_i           [128, y] psum via ones[128,128] lhsT (broadcast col-sums)
  A^T_i            [c, y] psum; normalized by 1/sums on PSUM->SBUF eviction
  O                [y, e] fp32
"""

import math
import sys

import numpy as np

sys.path.insert(0, "/opt/trn_rl_repo")

import ml_dtypes  # noqa: E402

import concourse.bass as bass  # noqa: E402
import concourse.mybir as mybir  # noqa: E402
from concourse import bacc  # noqa: E402
from concourse.bass_utils import run_bass_kernel_spmd  # noqa: E402
from concourse.tile import TileContext  # noqa: E402

F32 = mybir.dt.float32
BF16 = mybir.dt.bfloat16
EXP = mybir.ActivationFunctionType.Exp
COPY = mybir.ActivationFunctionType.Copy

B, S, E = 4, 1024, 256          # batch, seq, embed
HEADS = 8                        # total heads; each head dim = E (source quirk)
N_CORES = 8
HG_HEADS = 4                     # heads per head-group (per core)
NH = HG_HEADS * E                # projection columns per core = 1024
SCALE = 1.0 / math.sqrt(E // HEADS)   # 1/sqrt(32)

P = 128                          # SBUF partitions
NT = NH // P                     # 8 partition tiles of the projection dim
ST = S // P                      # 8 partition tiles of the seq dim
NCH = 512                        # matmul moving-dim chunk
SCH = S // NCH                   # 2 chunks of 512 over seq

_CACHE = {}

import os as _os
SCRATCH_BUFS = int(_os.environ.get("K_SCRATCH", "12"))
SC_BUFS = int(_os.environ.get("K_SC", "3"))
PA_BUFS = int(_os.environ.get("K_PA", "3"))
SR_BUFS = int(_os.environ.get("K_SR", "2"))


def _build():
    nc = bacc.Bacc(target_bir_lowering=False)

    XT = nc.dram_tensor("XT", [E, S], BF16, kind="ExternalInput")
    LT = nc.dram_tensor("LT", [E, S], BF16, kind="ExternalInput")
    WK = nc.dram_tensor("WK", [E, NH], BF16, kind="ExternalInput")
    WQ = nc.dram_tensor("WQ", [E, NH], BF16, kind="ExternalInput")
    WV = nc.dram_tensor("WV", [E, NH], BF16, kind="ExternalInput")
    WU = nc.dram_tensor("WU", [NH, E], BF16, kind="ExternalInput")
    O = nc.dram_tensor("O", [S, E], F32, kind="ExternalOutput")

    ET = E // P  # 2 partition tiles of the embed (contraction) dim

    with TileContext(nc) as tc:
        with tc.tile_pool(name="persist", bufs=1) as pp, \
             tc.tile_pool(name="scratch", bufs=SCRATCH_BUFS) as sp, \
             tc.tile_pool(name="small", bufs=2) as mp, \
             tc.tile_pool(name="psum", bufs=1, space="PSUM") as ps:

            ones = pp.tile([P, P], BF16, tag="ones", name="ones")
            nc.gpsimd.memset(ones[:], 1.0)

            # ---- phase 0: load inputs ----
            # wk/xt chunked so the first KT groups start early; wv/lt/wq as
            # single whole-tensor DMAs (fewer queue-issue slots, bigger lines)
            def alloc_in(nm):
                big = sp.tile([P, ET * S], BF16, tag="in", bufs=5,
                              name=f"{nm}a")
                return big, [big[:, t * S:(t + 1) * S] for t in range(ET)]

            (xta, xt), (wka, wk), (wva, wv), (lta, lt), (wqa, wq) = (
                alloc_in(n) for n in ("xt", "wk", "wv", "lt", "wq"))

            def dma_chunk(tiles, dram, e, c):
                nc.sync.dma_start(
                    out=tiles[e][:, c * NCH:(c + 1) * NCH],
                    in_=dram[e * P:(e + 1) * P, c * NCH:(c + 1) * NCH])

            def dma_whole(big, dram):
                nc.sync.dma_start(
                    out=big[:].rearrange("p (t s) -> p t s", t=ET),
                    in_=dram.rearrange("(t p) s -> p t s", p=P))

            for e in range(ET):
                nc.sync.dma_start(out=wk[e][:, 0:P],
                                  in_=WK[e * P:(e + 1) * P, 0:P])
            for e in range(ET):
                dma_chunk(xt, XT, e, 0)
            for e in range(ET):
                nc.sync.dma_start(out=wk[e][:, P:S],
                                  in_=WK[e * P:(e + 1) * P, P:S])
            for e in range(ET):
                dma_chunk(xt, XT, e, 1)
            dma_whole(wva, WV)
            dma_whole(lta, LT)
            dma_whole(wqa, WQ)
            wu_all = pp.tile([P, NT * E], BF16, tag="wu", name="wu_all")
            nc.sync.dma_start(out=wu_all[:].rearrange("p (t e) -> p t e", t=NT),
                              in_=WU.rearrange("(t p) e -> p t e", p=P))
            wu = [wu_all[:, t * E:(t + 1) * E] for t in range(NT)]

            # ---- phase 1: projections KT, QT (transposed), V (natural) ----
            kt, qt, v = [], [], []
            for nt in range(NT):
                kt.append(pp.tile([P, S], BF16, tag=f"kt{nt}", name=f"kt{nt}"))
                qt.append(pp.tile([P, S], BF16, tag=f"qt{nt}", name=f"qt{nt}"))
                v.append(pp.tile([P, S], BF16, tag=f"v{nt}", name=f"v{nt}"))

            # alternate PSUM->SBUF evictions between DVE and ACT so neither
            # engine gates the PE during the projection phase
            evict_ctr = [0]

            def evict(dst_ap, src_ap):
                evict_ctr[0] += 1
                if evict_ctr[0] % 2 == 0:
                    nc.vector.tensor_copy(dst_ap, src_ap)
                else:
                    nc.scalar.activation(dst_ap, src_ap, COPY)

            def proj(dst, lhs_tiles, lhs_cols, rhs_tiles, nt, c, nm):
                sl = bass.ts(c, NCH)
                pk = ps.tile([P, NCH], F32, tag="pa", bufs=PA_BUFS, name=f"p{nm}{nt}{c}")
                for e in range(ET):
                    nc.tensor.matmul(pk[:], lhs_tiles[e][:, lhs_cols],
                                     rhs_tiles[e][:, sl],
                                     start=(e == 0), stop=(e == ET - 1))
                evict(dst[nt][:, sl], pk[:])

            for c in range(SCH):
                for nt in range(NT):
                    proj(kt, wk, slice(nt * P, (nt + 1) * P), xt, nt, c, "k")
            for c in range(SCH):
                for nt in range(NT):
                    proj(v, xt, slice(nt * P, (nt + 1) * P), wv, nt, c, "v")
            for c in range(SCH):
                for nt in range(NT):
                    proj(qt, wq, slice(nt * P, (nt + 1) * P), lt, nt, c, "q")

            # ---- phase 2: attention per head ----
            at = []
            for nt in range(NT):
                at.append(pp.tile([P, S], BF16, tag=f"at{nt}", name=f"at{nt}"))

            pt_h = {}
            rec_h = {}
            srec = {}
            gs = {}

            # pending PE column-sum matmuls + reciprocal for a finished chunk;
            # emitted a few score-groups later so the PE (in-order queue)
            # never waits on the exp/add chain
            pend = []

            def emit_pending():
                if pend:
                    pend.pop(0)()

            def st_group(h, c, bt):
                # scores for one b-tile + exp; pre-reduce exp'd tiles pairwise
                # on GPSIMD/DVE so the chunk needs only 2 column-sum matmuls
                c0 = 2 * h
                sl = bass.ts(c, NCH)
                pt = pt_h[h]
                pss = ps.tile([P, NCH], F32, tag="sc", bufs=SC_BUFS,
                              name=f"pss{h}{bt}{c}")
                for cj in range(2):
                    nc.tensor.matmul(pss[:], kt[c0 + cj][:, bt * P:(bt + 1) * P],
                                     qt[c0 + cj][:, sl],
                                     start=(cj == 0), stop=(cj == 1))
                nc.scalar.activation(pt[bt][:, sl], pss[:], EXP, scale=SCALE)
                if bt <= 1:
                    emit_pending()
                g = gs[(h, c)]
                if bt == 1:
                    nc.gpsimd.tensor_add(g[0][:], pt[0][:, sl], pt[1][:, sl])
                elif bt == 3:
                    nc.gpsimd.tensor_add(g[1][:], pt[2][:, sl], pt[3][:, sl])
                elif bt == 4:
                    nc.gpsimd.tensor_add(g[2][:], g[0][:], g[1][:])
                elif bt == 5:
                    nc.vector.tensor_add(g[3][:], pt[4][:, sl], pt[5][:, sl])

            def finish_sums(h, c):
                sl = bass.ts(c, NCH)
                g = gs[(h, c)]
                nc.vector.tensor_add(g[4][:], pt_h[h][6][:, sl],
                                     pt_h[h][7][:, sl])
                nc.vector.tensor_add(g[4][:], g[3][:], g[4][:])

                def mm1(h=h, c=c):
                    nc.tensor.matmul(srec[(h, c)][:], ones[:], gs[(h, c)][2][:],
                                     start=True, stop=False)

                def mm2(h=h, c=c):
                    nc.tensor.matmul(srec[(h, c)][:], ones[:], gs[(h, c)][4][:],
                                     start=False, stop=True)
                    nc.vector.reciprocal_approx_fast(rec_h[(h, c)][:],
                                                     srec[(h, c)][:])
                pend.append(mm1)
                pend.append(mm2)

            def at_group(h, c, ct):
                # A^T accumulation over b; normalize on PSUM->SBUF eviction
                sl = bass.ts(c, NCH)
                vsl = slice(h * E + ct * P, h * E + (ct + 1) * P)
                pa = ps.tile([P, NCH], F32, tag="pa", bufs=PA_BUFS,
                             name=f"pa{h}{ct}{c}")
                for bt in range(ST):
                    nc.tensor.matmul(pa[:], v[bt][:, vsl], pt_h[h][bt][:, sl],
                                     start=(bt == 0), stop=(bt == ST - 1))
                nc.vector.tensor_mul(at[2 * h + ct][:, sl], pa[:],
                                     rec_h[(h, c)][:])

            def head_alloc(h):
                pt_h[h] = [sp.tile([P, S], BF16, tag="big", name=f"pt{h}{bt}")
                           for bt in range(ST)]
                for c in range(SCH):
                    rec_h[(h, c)] = mp.tile([P, NCH], F32, tag="rec", bufs=3,
                                            name=f"rec{h}{c}")
                    srec[(h, c)] = ps.tile([P, NCH], F32, tag="sr",
                                           bufs=SR_BUFS, name=f"sr{h}{c}")
                    gs[(h, c)] = [mp.tile([P, NCH], BF16, tag="gsum", bufs=12,
                                          name=f"g{h}{c}{i}")
                                  for i in range(5)]

            def out_tile(yt):
                po = ps.tile([P, E], F32, tag="sc", bufs=SC_BUFS,
                             name=f"po{yt}")
                for ht in range(NT):
                    nc.tensor.matmul(po[:], at[ht][:, yt * P:(yt + 1) * P],
                                     wu[ht],
                                     start=(ht == 0), stop=(ht == NT - 1))
                osb = mp.tile([P, E], F32, tag="osb", bufs=6, name=f"osb{yt}")
                evict(osb[:], po[:])
                nc.sync.dma_start(out=O[yt * P:(yt + 1) * P, :], in_=osb[:])

            # cross-head software pipeline: head h's chunk-1 A^T groups are
            # woven into head h+1's chunk-0 score stream, so the PE always has
            # matmul work while ACT drains the exp queue
            for h in range(HG_HEADS):
                head_alloc(h)
                for bt in range(ST):
                    st_group(h, 0, bt)
                    if h > 0:
                        if bt == 2:
                            at_group(h - 1, 1, 0)
                        elif bt == 5:
                            at_group(h - 1, 1, 1)
                finish_sums(h, 0)
                for bt in range(ST):
                    st_group(h, 1, bt)
                    if bt == 2:
                        at_group(h, 0, 0)
                    elif bt == 5:
                        at_group(h, 0, 1)
                finish_sums(h, 1)
            LAST = HG_HEADS - 1

            # ---- phase 3: output tiles woven with head-3's final A^T ----
            out_tile(0)
            emit_pending()
            out_tile(1)
            emit_pending()
            at_group(LAST, 1, 0)
            out_tile(2)
            out_tile(3)
            at_group(LAST, 1, 1)
            for yt in range(4, ST):
                out_tile(yt)

    nc.compile()
    return nc


def kernel(batch, latent, Wk, Wq, Wv, Wu, bu):
    bf16 = ml_dtypes.bfloat16
    batch = np.asarray(batch, dtype=np.float32)
    latent = np.asarray(latent, dtype=np.float32)
    bu = np.asarray(bu, dtype=np.float32)

    if "nc" not in _CACHE:
        _CACHE["nc"] = _build()
    nc = _CACHE["nc"]

    xts = [np.ascontiguousarray(batch[a].T).astype(bf16) for a in range(B)]
    lts = [np.ascontiguousarray(latent[a].T).astype(bf16) for a in range(B)]
    wks, wqs, wvs, wus = [], [], [], []
    for hg in range(2):
        cols = slice(hg * NH, (hg + 1) * NH)
        wks.append(np.ascontiguousarray(np.asarray(Wk, np.float32)[:, cols]).astype(bf16))
        wqs.append(np.ascontiguousarray(np.asarray(Wq, np.float32)[:, cols]).astype(bf16))
        wvs.append(np.ascontiguousarray(np.asarray(Wv, np.float32)[:, cols]).astype(bf16))
        wus.append(np.ascontiguousarray(np.asarray(Wu, np.float32)[cols, :]).astype(bf16))

    in_maps = []
    for core in range(N_CORES):
        a, hg = core // 2, core % 2
        in_maps.append({
            "XT": xts[a], "LT": lts[a],
            "WK": wks[hg], "WQ": wqs[hg], "WV": wvs[hg], "WU": wus[hg],
        })

    _CACHE["in_maps"] = in_maps
    res = run_bass_kernel_spmd(nc, in_maps, core_ids=list(range(N_CORES)))

    out = np.empty((B, S, E), dtype=np.float32)
    for a in range(B):
        out[a] = res.results[2 * a]["O"] + res.results[2 * a + 1]["O"] + bu
    return out


# revision 10
# speedup vs baseline: 2.0025x; 1.0019x over previous
"""Cross-attention kernel for 8 Trainium2 NeuronCores.

Reference computation (per batch element a):
  K = X @ Wk, Q = L @ Wq, V = X @ Wv          (each head uses a full 256-dim slice)
  S_i = Q_i @ K_i^T / sqrt(32); P = softmax(S); A_i = P_i @ V_i
  out = concat_i(A_i) @ Wu + bu
Sharding: core c = 2*a + hg handles batch a and head-group hg (4 heads, 1024
projection columns). The two partial outputs per batch element are summed on
the host (the "all-reduce after unify_heads"), which also adds the bias.

All operands are bf16 (PE runs 1 row/cycle, same as fp32r, but weight loads,
DMA and eviction bytes halve; measured end-to-end rel err ~4e-3 vs the 2e-2
gate). PSUM stays fp32.

Softmax denominators run on the PE: an all-ones [128,128] stationary tile
accumulates column sums of each exp'd P^T tile into a PSUM bank (producing the
sum broadcast across all 128 partitions directly), woven into the score
matmul stream.  A fast DVE reciprocal then yields 1/sum, and the PV-matmul
eviction multiplies it in on GPSIMD.  This removes the partition_all_reduce /
full-tile reciprocal / add-tree chain that serialized each head-chunk.

Device layouts (per core) -- contraction dim always on SBUF partitions:
  XT, LT           [256, 1024]  x^T / latent^T   (host pre-transposes)
  WK, WQ, WV       [256, 1024], WU [1024, 256]   natural
  KT = (X@WK)^T    [1024(n), 1024(s)] ; QT likewise ; V = X@WV [1024(s),1024(n)]
  S^T_i            [b, y] psum;  P^T_i = exp(.)  [b, y] sbuf bf16
  sums_i           [128, y] psum via ones[128,128] lhsT (broadcast col-sums)
  A^T_i            [c, y] psum; normalized by 1/sums on PSUM->SBUF eviction
  O                [y, e] fp32
"""

import math
import sys

import numpy as np

sys.path.insert(0, "/opt/trn_rl_repo")

import ml_dtypes  # noqa: E402

import concourse.bass as bass  # noqa: E402
import concourse.mybir as mybir  # noqa: E402
from concourse import bacc  # noqa: E402
from concourse.bass_utils import run_bass_kernel_spmd  # noqa: E402
from concourse.tile import TileContext  # noqa: E402

F32 = mybir.dt.float32
BF16 = mybir.dt.bfloat16
EXP = mybir.ActivationFunctionType.Exp
COPY = mybir.ActivationFunctionType.Copy

B, S, E = 4, 1024, 256          # batch, seq, embed
HEADS = 8                        # total heads; each head dim = E (source quirk)
N_CORES = 8
HG_HEADS = 4                     # heads per head-group (per core)
NH = HG_HEADS * E                # projection columns per core = 1024
SCALE = 1.0 / math.sqrt(E // HEADS)   # 1/sqrt(32)

P = 128                          # SBUF partitions
NT = NH // P                     # 8 partition tiles of the projection dim
ST = S // P                      # 8 partition tiles of the seq dim
NCH = 512                        # matmul moving-dim chunk
SCH = S // NCH                   # 2 chunks of 512 over seq

_CACHE = {}

import os as _os
SCRATCH_BUFS = int(_os.environ.get("K_SCRATCH", "12"))
SC_BUFS = int(_os.environ.get("K_SC", "3"))
PA_BUFS = int(_os.environ.get("K_PA", "3"))
SR_BUFS = int(_os.environ.get("K_SR", "2"))


def _build():
    nc = bacc.Bacc(target_bir_lowering=False)

    XT = nc.dram_tensor("XT", [E, S], BF16, kind="ExternalInput")
    LT = nc.dram_tensor("LT", [E, S], BF16, kind="ExternalInput")
    WK = nc.dram_tensor("WK", [E, NH], BF16, kind="ExternalInput")
    WQ = nc.dram_tensor("WQ", [E, NH], BF16, kind="ExternalInput")
    WV = nc.dram_tensor("WV", [E, NH], BF16, kind="ExternalInput")
    WU = nc.dram_tensor("WU", [NH, E], BF16, kind="ExternalInput")
    O = nc.dram_tensor("O", [S, E], F32, kind="ExternalOutput")

    ET = E // P  # 2 partition tiles of the embed (contraction) dim

    with TileContext(nc) as tc:
        with tc.tile_pool(name="persist", bufs=1) as pp, \
             tc.tile_pool(name="scratch", bufs=SCRATCH_BUFS) as sp, \
             tc.tile_pool(name="small", bufs=2) as mp, \
             tc.tile_pool(name="psum", bufs=1, space="PSUM") as ps:

            ones = pp.tile([P, P], BF16, tag="ones", name="ones")
            nc.gpsimd.memset(ones[:], 1.0)

            # ---- phase 0: load inputs ----
            # wk/xt chunked so the first KT groups start early; wv/lt/wq as
            # single whole-tensor DMAs (fewer queue-issue slots, bigger lines)
            def alloc_in(nm):
                big = sp.tile([P, ET * S], BF16, tag="in", bufs=5,
                              name=f"{nm}a")
                return big, [big[:, t * S:(t + 1) * S] for t in range(ET)]

            (xta, xt), (wka, wk), (wva, wv), (lta, lt), (wqa, wq) = (
                alloc_in(n) for n in ("xt", "wk", "wv", "lt", "wq"))

            def dma_pair(big, dram, c0, c1):
                # one DMA covering the same column range of both e-tiles
                nc.sync.dma_start(
                    out=big[:].rearrange("p (t s) -> p t s", t=ET)[:, :, c0:c1],
                    in_=dram.rearrange("(t p) s -> p t s", p=P)[:, :, c0:c1])

            def dma_whole(big, dram):
                nc.sync.dma_start(
                    out=big[:].rearrange("p (t s) -> p t s", t=ET),
                    in_=dram.rearrange("(t p) s -> p t s", p=P))

            dma_pair(wka, WK, 0, P)
            dma_pair(xta, XT, 0, NCH)
            dma_pair(wka, WK, P, S)
            dma_pair(xta, XT, NCH, S)
            dma_whole(wva, WV)
            dma_whole(lta, LT)
            dma_whole(wqa, WQ)
            wu_all = pp.tile([P, NT * E], BF16, tag="wu", name="wu_all")
            nc.sync.dma_start(out=wu_all[:].rearrange("p (t e) -> p t e", t=NT),
                              in_=WU.rearrange("(t p) e -> p t e", p=P))
            wu = [wu_all[:, t * E:(t + 1) * E] for t in range(NT)]

            # ---- phase 1: projections KT, QT (transposed), V (natural) ----
            kt, qt, v = [], [], []
            for nt in range(NT):
                kt.append(pp.tile([P, S], BF16, tag=f"kt{nt}", name=f"kt{nt}"))
                qt.append(pp.tile([P, S], BF16, tag=f"qt{nt}", name=f"qt{nt}"))
                v.append(pp.tile([P, S], BF16, tag=f"v{nt}", name=f"v{nt}"))

            # alternate PSUM->SBUF evictions between DVE and ACT so neither
            # engine gates the PE during the projection phase
            evict_ctr = [0]

            def evict(dst_ap, src_ap):
                evict_ctr[0] += 1
                if evict_ctr[0] % 2 == 0:
                    nc.vector.tensor_copy(dst_ap, src_ap)
                else:
                    nc.scalar.activation(dst_ap, src_ap, COPY)

            def proj(dst, lhs_tiles, lhs_cols, rhs_tiles, nt, c, nm):
                sl = bass.ts(c, NCH)
                pk = ps.tile([P, NCH], F32, tag="pa", bufs=PA_BUFS, name=f"p{nm}{nt}{c}")
                for e in range(ET):
                    nc.tensor.matmul(pk[:], lhs_tiles[e][:, lhs_cols],
                                     rhs_tiles[e][:, sl],
                                     start=(e == 0), stop=(e == ET - 1))
                evict(dst[nt][:, sl], pk[:])

            for c in range(SCH):
                for nt in range(NT):
                    proj(kt, wk, slice(nt * P, (nt + 1) * P), xt, nt, c, "k")
            for c in range(SCH):
                for nt in range(NT):
                    proj(v, xt, slice(nt * P, (nt + 1) * P), wv, nt, c, "v")
            for c in range(SCH):
                for nt in range(NT):
                    proj(qt, wq, slice(nt * P, (nt + 1) * P), lt, nt, c, "q")

            # ---- phase 2: attention per head ----
            at = []
            for nt in range(NT):
                at.append(pp.tile([P, S], BF16, tag=f"at{nt}", name=f"at{nt}"))

            pt_h = {}
            rec_h = {}
            srec = {}
            gs = {}

            # pending PE column-sum matmuls + reciprocal for a finished chunk;
            # emitted a few score-groups later so the PE (in-order queue)
            # never waits on the exp/add chain
            pend = []

            def emit_pending():
                if pend:
                    pend.pop(0)()

            def st_group(h, c, bt):
                # scores for one b-tile + exp; pre-reduce exp'd tiles pairwise
                # on GPSIMD/DVE so the chunk needs only 2 column-sum matmuls
                c0 = 2 * h
                sl = bass.ts(c, NCH)
                pt = pt_h[h]
                pss = ps.tile([P, NCH], F32, tag="sc", bufs=SC_BUFS,
                              name=f"pss{h}{bt}{c}")
                for cj in range(2):
                    nc.tensor.matmul(pss[:], kt[c0 + cj][:, bt * P:(bt + 1) * P],
                                     qt[c0 + cj][:, sl],
                                     start=(cj == 0), stop=(cj == 1))
                nc.scalar.activation(pt[bt][:, sl], pss[:], EXP, scale=SCALE)
                if bt <= 1:
                    emit_pending()
                g = gs[(h, c)]
                if bt == 1:
                    nc.gpsimd.tensor_add(g[0][:], pt[0][:, sl], pt[1][:, sl])
                elif bt == 3:
                    nc.gpsimd.tensor_add(g[1][:], pt[2][:, sl], pt[3][:, sl])
                elif bt == 4:
                    nc.vector.tensor_add(g[2][:], g[0][:], g[1][:])
                elif bt == 5:
                    nc.vector.tensor_add(g[3][:], pt[4][:, sl], pt[5][:, sl])

            def finish_sums(h, c):
                sl = bass.ts(c, NCH)
                g = gs[(h, c)]
                nc.vector.tensor_add(g[4][:], pt_h[h][6][:, sl],
                                     pt_h[h][7][:, sl])
                nc.vector.tensor_add(g[4][:], g[3][:], g[4][:])

                def mm1(h=h, c=c):
                    nc.tensor.matmul(srec[(h, c)][:], ones[:], gs[(h, c)][2][:],
                                     start=True, stop=False)

                def mm2(h=h, c=c):
                    nc.tensor.matmul(srec[(h, c)][:], ones[:], gs[(h, c)][4][:],
                                     start=False, stop=True)
                    nc.vector.reciprocal_approx_fast(rec_h[(h, c)][:],
                                                     srec[(h, c)][:])
                pend.append(mm1)
                pend.append(mm2)

            def at_group(h, c, ct):
                # A^T accumulation over b; normalize on PSUM->SBUF eviction
                sl = bass.ts(c, NCH)
                vsl = slice(h * E + ct * P, h * E + (ct + 1) * P)
                pa = ps.tile([P, NCH], F32, tag="pa", bufs=PA_BUFS,
                             name=f"pa{h}{ct}{c}")
                for bt in range(ST):
                    nc.tensor.matmul(pa[:], v[bt][:, vsl], pt_h[h][bt][:, sl],
                                     start=(bt == 0), stop=(bt == ST - 1))
                nc.vector.tensor_mul(at[2 * h + ct][:, sl], pa[:],
                                     rec_h[(h, c)][:])

            def head_alloc(h):
                pt_h[h] = [sp.tile([P, S], BF16, tag="big", name=f"pt{h}{bt}")
                           for bt in range(ST)]
                for c in range(SCH):
                    rec_h[(h, c)] = mp.tile([P, NCH], F32, tag="rec", bufs=3,
                                            name=f"rec{h}{c}")
                    srec[(h, c)] = ps.tile([P, NCH], F32, tag="sr",
                                           bufs=SR_BUFS, name=f"sr{h}{c}")
                    gs[(h, c)] = [mp.tile([P, NCH], BF16, tag="gsum", bufs=12,
                                          name=f"g{h}{c}{i}")
                                  for i in range(5)]

            def out_tile(yt):
                po = ps.tile([P, E], F32, tag="sc", bufs=SC_BUFS,
                             name=f"po{yt}")
                for ht in range(NT):
                    nc.tensor.matmul(po[:], at[ht][:, yt * P:(yt + 1) * P],
                                     wu[ht],
                                     start=(ht == 0), stop=(ht == NT - 1))
                osb = mp.tile([P, E], F32, tag="osb", bufs=6, name=f"osb{yt}")
                evict(osb[:], po[:])
                nc.sync.dma_start(out=O[yt * P:(yt + 1) * P, :], in_=osb[:])

            # cross-head software pipeline: head h's chunk-1 A^T groups are
            # woven into head h+1's chunk-0 score stream, so the PE always has
            # matmul work while ACT drains the exp queue
            for h in range(HG_HEADS):
                head_alloc(h)
                for bt in range(ST):
                    st_group(h, 0, bt)
                    if h > 0:
                        if bt == 2:
                            at_group(h - 1, 1, 0)
                        elif bt == 5:
                            at_group(h - 1, 1, 1)
                finish_sums(h, 0)
                for bt in range(ST):
                    st_group(h, 1, bt)
                    if bt == 2:
                        at_group(h, 0, 0)
                    elif bt == 5:
                        at_group(h, 0, 1)
                finish_sums(h, 1)
            LAST = HG_HEADS - 1

            # ---- phase 3: output tiles woven with head-3's final A^T ----
            out_tile(0)
            emit_pending()
            out_tile(1)
            emit_pending()
            at_group(LAST, 1, 0)
            out_tile(2)
            out_tile(3)
            at_group(LAST, 1, 1)
            for yt in range(4, ST):
                out_tile(yt)

    nc.compile()
    return nc


def kernel(batch, latent, Wk, Wq, Wv, Wu, bu):
    bf16 = ml_dtypes.bfloat16
    batch = np.asarray(batch, dtype=np.float32)
    latent = np.asarray(latent, dtype=np.float32)
    bu = np.asarray(bu, dtype=np.float32)

    if "nc" not in _CACHE:
        _CACHE["nc"] = _build()
    nc = _CACHE["nc"]

    xts = [np.ascontiguousarray(batch[a].T).astype(bf16) for a in range(B)]
    lts = [np.ascontiguousarray(latent[a].T).astype(bf16) for a in range(B)]
    wks, wqs, wvs, wus = [], [], [], []
    for hg in range(2):
        cols = slice(hg * NH, (hg + 1) * NH)
        wks.append(np.ascontiguousarray(np.asarray(Wk, np.float32)[:, cols]).astype(bf16))
        wqs.append(np.ascontiguousarray(np.asarray(Wq, np.float32)[:, cols]).astype(bf16))
        wvs.append(np.ascontiguousarray(np.asarray(Wv, np.float32)[:, cols]).astype(bf16))
        wus.append(np.ascontiguousarray(np.asarray(Wu, np.float32)[cols, :]).astype(bf16))

    in_maps = []
    for core in range(N_CORES):
        a, hg = core // 2, core % 2
        in_maps.append({
            "XT": xts[a], "LT": lts[a],
            "WK": wks[hg], "WQ": wqs[hg], "WV": wvs[hg], "WU": wus[hg],
        })

    _CACHE["in_maps"] = in_maps
    res = run_bass_kernel_spmd(nc, in_maps, core_ids=list(range(N_CORES)))

    out = np.empty((B, S, E), dtype=np.float32)
    for a in range(B):
        out[a] = res.results[2 * a]["O"] + res.results[2 * a + 1]["O"] + bu
    return out


# revision 13
# speedup vs baseline: 2.0304x; 1.0139x over previous
"""Cross-attention kernel for 8 Trainium2 NeuronCores.

Reference computation (per batch element a):
  K = X @ Wk, Q = L @ Wq, V = X @ Wv          (each head uses a full 256-dim slice)
  S_i = Q_i @ K_i^T / sqrt(32); P = softmax(S); A_i = P_i @ V_i
  out = concat_i(A_i) @ Wu + bu
Sharding: core c = 2*a + hg handles batch a and head-group hg (4 heads, 1024
projection columns). The two partial outputs per batch element are summed on
the host (the "all-reduce after unify_heads"), which also adds the bias.

All operands are bf16 (PE runs 1 row/cycle, same as fp32r, but weight loads,
DMA and eviction bytes halve; measured end-to-end rel err ~4e-3 vs the 2e-2
gate). PSUM stays fp32.

Softmax denominators run on the PE: an all-ones [128,128] stationary tile
accumulates column sums of each exp'd P^T tile into a PSUM bank (producing the
sum broadcast across all 128 partitions directly), woven into the score
matmul stream.  A fast DVE reciprocal then yields 1/sum, and the PV-matmul
eviction multiplies it in on GPSIMD.  This removes the partition_all_reduce /
full-tile reciprocal / add-tree chain that serialized each head-chunk.

Device layouts (per core) -- contraction dim always on SBUF partitions:
  XT, LT           [256, 1024]  x^T / latent^T   (host pre-transposes)
  WK, WQ, WV       [256, 1024], WU [1024, 256]   natural
  KT = (X@WK)^T    [1024(n), 1024(s)] ; QT likewise ; V = X@WV [1024(s),1024(n)]
  S^T_i            [b, y] psum;  P^T_i = exp(.)  [b, y] sbuf bf16
  sums_i           [128, y] psum via ones[128,128] lhsT (broadcast col-sums)
  A^T_i            [c, y] psum; normalized by 1/sums on PSUM->SBUF eviction
  O                [y, e] fp32
"""

import math
import sys

import numpy as np

sys.path.insert(0, "/opt/trn_rl_repo")

import ml_dtypes  # noqa: E402

import concourse.bass as bass  # noqa: E402
import concourse.mybir as mybir  # noqa: E402
from concourse import bacc  # noqa: E402
from concourse.bass_utils import run_bass_kernel_spmd  # noqa: E402
from concourse.tile import TileContext  # noqa: E402

F32 = mybir.dt.float32
BF16 = mybir.dt.bfloat16
EXP = mybir.ActivationFunctionType.Exp
COPY = mybir.ActivationFunctionType.Copy

B, S, E = 4, 1024, 256          # batch, seq, embed
HEADS = 8                        # total heads; each head dim = E (source quirk)
N_CORES = 8
HG_HEADS = 4                     # heads per head-group (per core)
NH = HG_HEADS * E                # projection columns per core = 1024
SCALE = 1.0 / math.sqrt(E // HEADS)   # 1/sqrt(32)

P = 128                          # SBUF partitions
NT = NH // P                     # 8 partition tiles of the projection dim
ST = S // P                      # 8 partition tiles of the seq dim
NCH = 512                        # matmul moving-dim chunk
SCH = S // NCH                   # 2 chunks of 512 over seq

_CACHE = {}

import os as _os
SCRATCH_BUFS = int(_os.environ.get("K_SCRATCH", "12"))
SC_BUFS = int(_os.environ.get("K_SC", "4"))
PA_BUFS = int(_os.environ.get("K_PA", "3"))
SR_BUFS = int(_os.environ.get("K_SR", "1"))


def _build():
    nc = bacc.Bacc(target_bir_lowering=False)

    XT = nc.dram_tensor("XT", [E, S], BF16, kind="ExternalInput")
    LT = nc.dram_tensor("LT", [E, S], BF16, kind="ExternalInput")
    WK = nc.dram_tensor("WK", [E, NH], BF16, kind="ExternalInput")
    WQ = nc.dram_tensor("WQ", [E, NH], BF16, kind="ExternalInput")
    WV = nc.dram_tensor("WV", [E, NH], BF16, kind="ExternalInput")
    WU = nc.dram_tensor("WU", [NH, E], BF16, kind="ExternalInput")
    O = nc.dram_tensor("O", [S, E], F32, kind="ExternalOutput")

    ET = E // P  # 2 partition tiles of the embed (contraction) dim

    with TileContext(nc) as tc:
        with tc.tile_pool(name="persist", bufs=1) as pp, \
             tc.tile_pool(name="scratch", bufs=SCRATCH_BUFS) as sp, \
             tc.tile_pool(name="small", bufs=2) as mp, \
             tc.tile_pool(name="psum", bufs=1, space="PSUM") as ps:

            ones = pp.tile([P, P], BF16, tag="ones", name="ones")
            nc.gpsimd.memset(ones[:], 1.0)

            # ---- phase 0: load inputs ----
            # wk/xt chunked so the first KT groups start early; wv/lt/wq as
            # single whole-tensor DMAs (fewer queue-issue slots, bigger lines)
            def alloc_in(nm):
                big = sp.tile([P, ET * S], BF16, tag="in", bufs=5,
                              name=f"{nm}a")
                return big, [big[:, t * S:(t + 1) * S] for t in range(ET)]

            (xta, xt), (wka, wk), (wva, wv), (lta, lt), (wqa, wq) = (
                alloc_in(n) for n in ("xt", "wk", "wv", "lt", "wq"))

            def dma_pair(eng, big, dram, c0, c1):
                # one DMA covering the same column range of both e-tiles
                eng.dma_start(
                    out=big[:].rearrange("p (t s) -> p t s", t=ET)[:, :, c0:c1],
                    in_=dram.rearrange("(t p) s -> p t s", p=P)[:, :, c0:c1])

            # two HWDGE queues (sync + scalar) so the first KT group's
            # operands transfer in parallel
            dma_pair(nc.sync, wka, WK, 0, P)
            dma_pair(nc.scalar, xta, XT, 0, NCH)
            dma_pair(nc.sync, wka, WK, P, S)
            dma_pair(nc.scalar, xta, XT, NCH, S)
            dma_pair(nc.scalar, wva, WV, 0, S)
            dma_pair(nc.sync, lta, LT, 0, S)
            dma_pair(nc.sync, wqa, WQ, 0, S)
            wu_all = pp.tile([P, NT * E], BF16, tag="wu", name="wu_all")
            nc.sync.dma_start(out=wu_all[:].rearrange("p (t e) -> p t e", t=NT),
                              in_=WU.rearrange("(t p) e -> p t e", p=P))
            wu = [wu_all[:, t * E:(t + 1) * E] for t in range(NT)]

            # ---- phase 1: projections KT, QT (transposed), V (natural) ----
            kt, qt, v = [], [], []
            for nt in range(NT):
                kt.append(pp.tile([P, S], BF16, tag=f"kt{nt}", name=f"kt{nt}"))
                qt.append(pp.tile([P, S], BF16, tag=f"qt{nt}", name=f"qt{nt}"))
                v.append(pp.tile([P, S], BF16, tag=f"v{nt}", name=f"v{nt}"))

            # alternate PSUM->SBUF evictions between DVE and ACT so neither
            # engine gates the PE during the projection phase
            evict_ctr = [0]

            def evict(dst_ap, src_ap):
                evict_ctr[0] += 1
                if evict_ctr[0] % 2 == 0:
                    nc.vector.tensor_copy(dst_ap, src_ap)
                else:
                    nc.scalar.activation(dst_ap, src_ap, COPY)

            def proj(dst, lhs_tiles, lhs_cols, rhs_tiles, nt, c, nm):
                sl = bass.ts(c, NCH)
                pk = ps.tile([P, NCH], F32, tag="pa", bufs=PA_BUFS, name=f"p{nm}{nt}{c}")
                for e in range(ET):
                    nc.tensor.matmul(pk[:], lhs_tiles[e][:, lhs_cols],
                                     rhs_tiles[e][:, sl],
                                     start=(e == 0), stop=(e == ET - 1))
                evict(dst[nt][:, sl], pk[:])

            for c in range(SCH):
                for nt in range(NT):
                    proj(kt, wk, slice(nt * P, (nt + 1) * P), xt, nt, c, "k")
            for c in range(SCH):
                for nt in range(NT):
                    proj(v, xt, slice(nt * P, (nt + 1) * P), wv, nt, c, "v")
            for c in range(SCH):
                for nt in range(NT):
                    proj(qt, wq, slice(nt * P, (nt + 1) * P), lt, nt, c, "q")

            # ---- phase 2: attention per head ----
            at = []
            for nt in range(NT):
                at.append(pp.tile([P, S], BF16, tag=f"at{nt}", name=f"at{nt}"))

            pt_h = {}
            rec_h = {}
            srec = {}
            gs = {}

            # pending PE column-sum matmuls + reciprocal for a finished chunk;
            # emitted a few score-groups later so the PE (in-order queue)
            # never waits on the exp/add chain
            pend = []

            def emit_pending():
                if pend:
                    pend.pop(0)()

            def st_group(h, c, bt):
                # scores for one b-tile + exp; pre-reduce exp'd tiles pairwise
                # on GPSIMD/DVE so the chunk needs only 2 column-sum matmuls
                c0 = 2 * h
                sl = bass.ts(c, NCH)
                pt = pt_h[h]
                pss = ps.tile([P, NCH], F32, tag="sc", bufs=SC_BUFS,
                              name=f"pss{h}{bt}{c}")
                for cj in range(2):
                    nc.tensor.matmul(pss[:], kt[c0 + cj][:, bt * P:(bt + 1) * P],
                                     qt[c0 + cj][:, sl],
                                     start=(cj == 0), stop=(cj == 1))
                nc.scalar.activation(pt[bt][:, sl], pss[:], EXP, scale=SCALE)
                if bt <= 1:
                    emit_pending()
                g = gs[(h, c)]
                if bt == 1:
                    nc.gpsimd.tensor_add(g[0][:], pt[0][:, sl], pt[1][:, sl])
                elif bt == 3:
                    nc.vector.tensor_add(g[1][:], pt[2][:, sl], pt[3][:, sl])
                elif bt == 4:
                    nc.vector.tensor_add(g[2][:], g[0][:], g[1][:])
                elif bt == 5:
                    nc.vector.tensor_add(g[3][:], pt[4][:, sl], pt[5][:, sl])

            def finish_sums(h, c):
                sl = bass.ts(c, NCH)
                g = gs[(h, c)]
                nc.vector.tensor_add(g[4][:], pt_h[h][6][:, sl],
                                     pt_h[h][7][:, sl])
                nc.vector.tensor_add(g[4][:], g[3][:], g[4][:])

                def mm1(h=h, c=c):
                    nc.tensor.matmul(srec[(h, c)][:], ones[:], gs[(h, c)][2][:],
                                     start=True, stop=False)

                def mm2(h=h, c=c):
                    nc.tensor.matmul(srec[(h, c)][:], ones[:], gs[(h, c)][4][:],
                                     start=False, stop=True)
                    nc.vector.reciprocal_approx_fast(rec_h[(h, c)][:],
                                                     srec[(h, c)][:])
                pend.append(mm1)
                pend.append(mm2)

            def at_group(h, c, ct):
                # A^T accumulation over b; normalize on PSUM->SBUF eviction
                sl = bass.ts(c, NCH)
                vsl = slice(h * E + ct * P, h * E + (ct + 1) * P)
                pa = ps.tile([P, NCH], F32, tag="pa", bufs=PA_BUFS,
                             name=f"pa{h}{ct}{c}")
                for bt in range(ST):
                    nc.tensor.matmul(pa[:], v[bt][:, vsl], pt_h[h][bt][:, sl],
                                     start=(bt == 0), stop=(bt == ST - 1))
                nc.vector.tensor_mul(at[2 * h + ct][:, sl], pa[:],
                                     rec_h[(h, c)][:])

            def head_alloc(h):
                pt_h[h] = [sp.tile([P, S], BF16, tag="big", name=f"pt{h}{bt}")
                           for bt in range(ST)]
                for c in range(SCH):
                    rec_h[(h, c)] = mp.tile([P, NCH], F32, tag="rec", bufs=3,
                                            name=f"rec{h}{c}")
                    srec[(h, c)] = ps.tile([P, NCH], F32, tag="sr",
                                           bufs=SR_BUFS, name=f"sr{h}{c}")
                    gs[(h, c)] = [mp.tile([P, NCH], BF16, tag="gsum", bufs=12,
                                          name=f"g{h}{c}{i}")
                                  for i in range(5)]

            def out_tile(yt):
                po = ps.tile([P, E], F32, tag="sc", bufs=SC_BUFS,
                             name=f"po{yt}")
                for ht in range(NT):
                    nc.tensor.matmul(po[:], at[ht][:, yt * P:(yt + 1) * P],
                                     wu[ht],
                                     start=(ht == 0), stop=(ht == NT - 1))
                osb = mp.tile([P, E], F32, tag="osb", bufs=6, name=f"osb{yt}")
                evict(osb[:], po[:])
                nc.sync.dma_start(out=O[yt * P:(yt + 1) * P, :], in_=osb[:])

            # cross-head software pipeline: head h's chunk-1 A^T groups are
            # woven into head h+1's chunk-0 score stream, so the PE always has
            # matmul work while ACT drains the exp queue
            for h in range(HG_HEADS):
                head_alloc(h)
                for bt in range(ST):
                    st_group(h, 0, bt)
                    if h > 0:
                        if bt == 2:
                            at_group(h - 1, 1, 0)
                        elif bt == 5:
                            at_group(h - 1, 1, 1)
                finish_sums(h, 0)
                for bt in range(ST):
                    st_group(h, 1, bt)
                    if bt == 2:
                        at_group(h, 0, 0)
                    elif bt == 5:
                        at_group(h, 0, 1)
                finish_sums(h, 1)
            LAST = HG_HEADS - 1

            # ---- phase 3: output tiles woven with head-3's final A^T ----
            out_tile(0)
            emit_pending()
            out_tile(1)
            emit_pending()
            at_group(LAST, 1, 0)
            out_tile(2)
            out_tile(3)
            at_group(LAST, 1, 1)
            for yt in range(4, ST):
                out_tile(yt)

    nc.compile()
    return nc


def kernel(batch, latent, Wk, Wq, Wv, Wu, bu):
    bf16 = ml_dtypes.bfloat16
    batch = np.asarray(batch, dtype=np.float32)
    latent = np.asarray(latent, dtype=np.float32)
    bu = np.asarray(bu, dtype=np.float32)

    if "nc" not in _CACHE:
        _CACHE["nc"] = _build()
    nc = _CACHE["nc"]

    xts = [np.ascontiguousarray(batch[a].T).astype(bf16) for a in range(B)]
    lts = [np.ascontiguousarray(latent[a].T).astype(bf16) for a in range(B)]
    wks, wqs, wvs, wus = [], [], [], []
    for hg in range(2):
        cols = slice(hg * NH, (hg + 1) * NH)
        wks.append(np.ascontiguousarray(np.asarray(Wk, np.float32)[:, cols]).astype(bf16))
        wqs.append(np.ascontiguousarray(np.asarray(Wq, np.float32)[:, cols]).astype(bf16))
        wvs.append(np.ascontiguousarray(np.asarray(Wv, np.float32)[:, cols]).astype(bf16))
        wus.append(np.ascontiguousarray(np.asarray(Wu, np.float32)[cols, :]).astype(bf16))

    in_maps = []
    for core in range(N_CORES):
        a, hg = core // 2, core % 2
        in_maps.append({
            "XT": xts[a], "LT": lts[a],
            "WK": wks[hg], "WQ": wqs[hg], "WV": wvs[hg], "WU": wus[hg],
        })

    _CACHE["in_maps"] = in_maps
    res = run_bass_kernel_spmd(nc, in_maps, core_ids=list(range(N_CORES)))

    out = np.empty((B, S, E), dtype=np.float32)
    for a in range(B):
        out[a] = res.results[2 * a]["O"] + res.results[2 * a + 1]["O"] + bu
    return out


# revision 14
# speedup vs baseline: 2.1710x; 1.0693x over previous
"""Cross-attention kernel for 8 Trainium2 NeuronCores.

Reference computation (per batch element a):
  K = X @ Wk, Q = L @ Wq, V = X @ Wv          (each head uses a full 256-dim slice)
  S_i = Q_i @ K_i^T / sqrt(32); P = softmax(S); A_i = P_i @ V_i
  out = concat_i(A_i) @ Wu + bu
Sharding: core c = 2*a + hg handles batch a and head-group hg (4 heads).  The
two partial outputs per batch element are summed on the host (the "all-reduce
after unify_heads"), which also adds the bias.

Algebraic fusion (per head i): Q_i K_i^T = L (Wq_i Wk_i^T) X^T, so with
G_i := Wk_i Wq_i^T (256x256) one projection K' = X @ G_i replaces both the K
and Q projections, and L^T is used directly as the score matmul's moving
operand.  Likewise A_i @ Wu_i = (P_i X)(Wv_i Wu_i) =: PX @ N_i with
N_i := Wv_i @ Wu_i, eliminating the V projection.  G/N are tiny on-device
precomputes from host-transposed weights.

All operands are bf16 (PE runs 1 row/cycle, same as fp32r, but weight loads,
DMA and eviction bytes halve; measured end-to-end rel err ~4.5e-3 vs the 2e-2
gate).  PSUM stays fp32.

Softmax denominators: exp'd P^T tiles are pre-reduced pairwise on GPSIMD/DVE,
then an all-ones [128,128] stationary tile column-sums the two partial tiles
into PSUM (sum broadcast across partitions in one matmul), woven into the
next chunk's score stream so the in-order PE queue never waits.  A fast DVE
reciprocal yields 1/sum; the PX eviction multiplies it in.

Device layouts (per core) -- contraction dim always on SBUF partitions:
  XT, LT             [256, 1024]  x^T / latent^T  (host pre-transposes)
  XN                 [1024, 256]  x natural       (PX lhsT)
  WKT, WQT, WVT      [1024, 256]  transposed weights; WU [1024, 256] natural
  G_i = Wk_i Wq_i^T  [e, f] 2 tiles/head;  N_i = Wv_i Wu_i [e, e''] 2/head
  K'T = (X@G)^T      [1024(f), 1024(b)] ; S^T_i [b, y] psum
  P^T_i = exp(.)     [b, y] sbuf bf16; sums via ones-matmul -> psum
  PX^T_i             [e, y] psum; normalized by 1/sums on eviction
  O                  [y, e] fp32
"""

import math
import sys

import numpy as np

sys.path.insert(0, "/opt/trn_rl_repo")

import ml_dtypes  # noqa: E402

import concourse.bass as bass  # noqa: E402
import concourse.mybir as mybir  # noqa: E402
from concourse import bacc  # noqa: E402
from concourse.bass_utils import run_bass_kernel_spmd  # noqa: E402
from concourse.tile import TileContext  # noqa: E402

F32 = mybir.dt.float32
BF16 = mybir.dt.bfloat16
EXP = mybir.ActivationFunctionType.Exp
COPY = mybir.ActivationFunctionType.Copy

B, S, E = 4, 1024, 256          # batch, seq, embed
HEADS = 8                        # total heads; each head dim = E (source quirk)
N_CORES = 8
HG_HEADS = 4                     # heads per head-group (per core)
NH = HG_HEADS * E                # projection columns per core = 1024
SCALE = 1.0 / math.sqrt(E // HEADS)   # 1/sqrt(32)

P = 128                          # SBUF partitions
NT = NH // P                     # 8 partition tiles of the projection dim
ST = S // P                      # 8 partition tiles of the seq dim
NCH = 512                        # matmul moving-dim chunk
SCH = S // NCH                   # 2 chunks of 512 over seq

_CACHE = {}

import os as _os
SCRATCH_BUFS = int(_os.environ.get("K_SCRATCH", "12"))
SC_BUFS = int(_os.environ.get("K_SC", "4"))
PA_BUFS = int(_os.environ.get("K_PA", "3"))
SR_BUFS = int(_os.environ.get("K_SR", "1"))


def _build():
    nc = bacc.Bacc(target_bir_lowering=False)

    XT = nc.dram_tensor("XT", [E, S], BF16, kind="ExternalInput")
    XND = nc.dram_tensor("XN", [S, E], BF16, kind="ExternalInput")
    LT = nc.dram_tensor("LT", [E, S], BF16, kind="ExternalInput")
    WKT = nc.dram_tensor("WKT", [NH, E], BF16, kind="ExternalInput")
    WQT = nc.dram_tensor("WQT", [NH, E], BF16, kind="ExternalInput")
    WVT = nc.dram_tensor("WVT", [NH, E], BF16, kind="ExternalInput")
    WU = nc.dram_tensor("WU", [NH, E], BF16, kind="ExternalInput")
    O = nc.dram_tensor("O", [S, E], F32, kind="ExternalOutput")

    ET = E // P  # 2 partition tiles of the embed (contraction) dim

    with TileContext(nc) as tc:
        with tc.tile_pool(name="persist", bufs=1) as pp, \
             tc.tile_pool(name="scratch", bufs=SCRATCH_BUFS) as sp, \
             tc.tile_pool(name="small", bufs=2) as mp, \
             tc.tile_pool(name="psum", bufs=1, space="PSUM") as ps:

            ones = pp.tile([P, P], BF16, tag="ones", name="ones")
            nc.gpsimd.memset(ones[:], 1.0)

            # ---- phase 0: load inputs over both HWDGE queues ----
            def alloc_in(nm, cols):
                big = sp.tile([P, cols], BF16, tag="in", bufs=6, name=f"{nm}a")
                return big

            xta = alloc_in("xt", ET * S)
            lta = alloc_in("lt", ET * S)
            xna = alloc_in("xn", ST * E)
            wkta = alloc_in("wkt", NT * E)
            wqta = alloc_in("wqt", NT * E)
            wvta = alloc_in("wvt", NT * E)
            xt = [xta[:, t * S:(t + 1) * S] for t in range(ET)]
            lt = [lta[:, t * S:(t + 1) * S] for t in range(ET)]
            xn = [xna[:, t * E:(t + 1) * E] for t in range(ST)]
            wkt = [wkta[:, t * E:(t + 1) * E] for t in range(NT)]
            wqt = [wqta[:, t * E:(t + 1) * E] for t in range(NT)]
            wvt = [wvta[:, t * E:(t + 1) * E] for t in range(NT)]

            def dma_in(eng, big, dram, nt_, c0, c1):
                eng.dma_start(
                    out=big[:].rearrange("p (t s) -> p t s", t=nt_)[:, :, c0:c1],
                    in_=dram.rearrange("(t p) s -> p t s", p=P)[:, :, c0:c1])

            dma_in(nc.sync, wkta, WKT, NT, 0, E)
            dma_in(nc.scalar, wqta, WQT, NT, 0, E)
            dma_in(nc.sync, xta, XT, ET, 0, NCH)
            dma_in(nc.scalar, lta, LT, ET, 0, S)
            dma_in(nc.sync, xta, XT, ET, NCH, S)
            dma_in(nc.scalar, xna, XND, ST, 0, E)
            wu_all = pp.tile([P, NT * E], BF16, tag="wu", name="wu_all")
            nc.sync.dma_start(out=wu_all[:].rearrange("p (t e) -> p t e", t=NT),
                              in_=WU.rearrange("(t p) e -> p t e", p=P))
            dma_in(nc.scalar, wvta, WVT, NT, 0, E)
            wu = [wu_all[:, t * E:(t + 1) * E] for t in range(NT)]

            # alternate PSUM->SBUF evictions between DVE and ACT
            evict_ctr = [0]

            def evict(dst_ap, src_ap):
                evict_ctr[0] += 1
                if evict_ctr[0] % 2 == 0:
                    nc.vector.tensor_copy(dst_ap, src_ap)
                else:
                    nc.scalar.activation(dst_ap, src_ap, COPY)

            # ---- phase 1a: G_i = Wk_i Wq_i^T  (tiles G[e(part),f]) ----
            mth = pp.tile([P, HG_HEADS * ET * E], BF16, tag="mt", name="mt")
            mt = [[mth[:, (2 * h + et) * E:(2 * h + et + 1) * E]
                   for et in range(ET)] for h in range(HG_HEADS)]
            nth = pp.tile([P, HG_HEADS * ET * E], BF16, tag="nt", name="nt")
            ntv = [nth[:, t * E:(t + 1) * E] for t in range(NT)]

            def weight_prod(dst, lhs_t, rhs_t, h, et, nm):
                pm = ps.tile([P, E], F32, tag="pa", bufs=PA_BUFS,
                             name=f"p{nm}{h}{et}")
                for nk in range(2):
                    nc.tensor.matmul(pm[:],
                                     lhs_t[2 * h + nk][:, et * P:(et + 1) * P],
                                     rhs_t[2 * h + nk][:],
                                     start=(nk == 0), stop=(nk == 1))
                evict(dst[:], pm[:])

            for h in range(HG_HEADS):
                for et in range(ET):
                    weight_prod(mt[h][et], wkt, wqt, h, et, "m")

            # ---- phase 1b: K'T projection per head ----
            kt = []
            for nt_ in range(NT):
                kt.append(pp.tile([P, S], BF16, tag=f"kt{nt_}", name=f"kt{nt_}"))

            def proj(h, ct, c):
                sl = bass.ts(c, NCH)
                pk = ps.tile([P, NCH], F32, tag="pa", bufs=PA_BUFS,
                             name=f"pk{h}{ct}{c}")
                for ek in range(ET):
                    nc.tensor.matmul(pk[:],
                                     mt[h][ek][:, ct * P:(ct + 1) * P],
                                     xt[ek][:, sl],
                                     start=(ek == 0), stop=(ek == ET - 1))
                evict(kt[2 * h + ct][:, sl], pk[:])

            for h in range(HG_HEADS):
                for c in range(SCH):
                    for ct in range(ET):
                        proj(h, ct, c)
                if h == 0:
                    # N_i = Wv_i Wu_i, woven in while LT still streams
                    for hh in range(HG_HEADS):
                        for et in range(ET):
                            weight_prod(ntv[2 * hh + et], wvt, wu, hh, et, "n")

            # ---- phase 2: attention per head ----
            pxt = []
            for nt_ in range(NT):
                pxt.append(pp.tile([P, S], BF16, tag=f"px{nt_}",
                                   name=f"px{nt_}"))

            pt_h = {}
            rec_h = {}
            srec = {}
            gs = {}

            # pending PE column-sum matmuls + reciprocal for a finished chunk;
            # emitted a few score-groups later so the PE (in-order queue)
            # never waits on the exp/add chain
            pend = []

            def emit_pending():
                if pend:
                    pend.pop(0)()

            def st_group(h, c, bt):
                # scores for one b-tile + exp; pre-reduce exp'd tiles pairwise
                # on GPSIMD/DVE so the chunk needs only 2 column-sum matmuls
                c0 = 2 * h
                sl = bass.ts(c, NCH)
                pt = pt_h[h]
                pss = ps.tile([P, NCH], F32, tag="sc", bufs=SC_BUFS,
                              name=f"pss{h}{bt}{c}")
                for cj in range(2):
                    nc.tensor.matmul(pss[:], kt[c0 + cj][:, bt * P:(bt + 1) * P],
                                     lt[cj][:, sl],
                                     start=(cj == 0), stop=(cj == 1))
                nc.scalar.activation(pt[bt][:, sl], pss[:], EXP, scale=SCALE)
                if bt <= 1:
                    emit_pending()
                g = gs[(h, c)]
                if bt == 1:
                    nc.gpsimd.tensor_add(g[0][:], pt[0][:, sl], pt[1][:, sl])
                elif bt == 3:
                    nc.vector.tensor_add(g[1][:], pt[2][:, sl], pt[3][:, sl])
                elif bt == 4:
                    nc.vector.tensor_add(g[2][:], g[0][:], g[1][:])
                elif bt == 5:
                    nc.vector.tensor_add(g[3][:], pt[4][:, sl], pt[5][:, sl])

            def finish_sums(h, c):
                sl = bass.ts(c, NCH)
                g = gs[(h, c)]
                nc.vector.tensor_add(g[4][:], pt_h[h][6][:, sl],
                                     pt_h[h][7][:, sl])
                nc.vector.tensor_add(g[4][:], g[3][:], g[4][:])

                def mm1(h=h, c=c):
                    nc.tensor.matmul(srec[(h, c)][:], ones[:], gs[(h, c)][2][:],
                                     start=True, stop=False)

                def mm2(h=h, c=c):
                    nc.tensor.matmul(srec[(h, c)][:], ones[:], gs[(h, c)][4][:],
                                     start=False, stop=True)
                    nc.vector.reciprocal_approx_fast(rec_h[(h, c)][:],
                                                     srec[(h, c)][:])
                pend.append(mm1)
                pend.append(mm2)

            def at_group(h, c, ct):
                # PX^T accumulation over b; normalize on PSUM->SBUF eviction
                sl = bass.ts(c, NCH)
                pa = ps.tile([P, NCH], F32, tag="pa", bufs=PA_BUFS,
                             name=f"pa{h}{ct}{c}")
                for bt in range(ST):
                    nc.tensor.matmul(pa[:], xn[bt][:, ct * P:(ct + 1) * P],
                                     pt_h[h][bt][:, sl],
                                     start=(bt == 0), stop=(bt == ST - 1))
                nc.vector.tensor_mul(pxt[2 * h + ct][:, sl], pa[:],
                                     rec_h[(h, c)][:])

            def head_alloc(h):
                pt_h[h] = [sp.tile([P, S], BF16, tag="big", name=f"pt{h}{bt}")
                           for bt in range(ST)]
                for c in range(SCH):
                    rec_h[(h, c)] = mp.tile([P, NCH], F32, tag="rec", bufs=3,
                                            name=f"rec{h}{c}")
                    srec[(h, c)] = ps.tile([P, NCH], F32, tag="sr",
                                           bufs=SR_BUFS, name=f"sr{h}{c}")
                    gs[(h, c)] = [mp.tile([P, NCH], BF16, tag="gsum", bufs=12,
                                          name=f"g{h}{c}{i}")
                                  for i in range(5)]

            def out_tile(yt):
                po = ps.tile([P, E], F32, tag="sc", bufs=SC_BUFS,
                             name=f"po{yt}")
                for ht in range(NT):
                    nc.tensor.matmul(po[:], pxt[ht][:, yt * P:(yt + 1) * P],
                                     ntv[ht],
                                     start=(ht == 0), stop=(ht == NT - 1))
                osb = mp.tile([P, E], F32, tag="osb", bufs=6, name=f"osb{yt}")
                evict(osb[:], po[:])
                nc.sync.dma_start(out=O[yt * P:(yt + 1) * P, :], in_=osb[:])

            # cross-head software pipeline: head h's chunk-1 PX^T groups are
            # woven into head h+1's chunk-0 score stream, so the PE always has
            # matmul work while ACT drains the exp queue
            for h in range(HG_HEADS):
                head_alloc(h)
                for bt in range(ST):
                    st_group(h, 0, bt)
                    if h > 0:
                        if bt == 2:
                            at_group(h - 1, 1, 0)
                        elif bt == 5:
                            at_group(h - 1, 1, 1)
                finish_sums(h, 0)
                for bt in range(ST):
                    st_group(h, 1, bt)
                    if bt == 2:
                        at_group(h, 0, 0)
                    elif bt == 5:
                        at_group(h, 0, 1)
                finish_sums(h, 1)
            LAST = HG_HEADS - 1

            # ---- phase 3: output tiles woven with head-3's final PX^T ----
            out_tile(0)
            emit_pending()
            out_tile(1)
            emit_pending()
            at_group(LAST, 1, 0)
            out_tile(2)
            out_tile(3)
            at_group(LAST, 1, 1)
            for yt in range(4, ST):
                out_tile(yt)

    nc.compile()
    return nc


def kernel(batch, latent, Wk, Wq, Wv, Wu, bu):
    bf16 = ml_dtypes.bfloat16
    batch = np.asarray(batch, dtype=np.float32)
    latent = np.asarray(latent, dtype=np.float32)
    bu = np.asarray(bu, dtype=np.float32)

    if "nc" not in _CACHE:
        _CACHE["nc"] = _build()
    nc = _CACHE["nc"]

    xts = [np.ascontiguousarray(batch[a].T).astype(bf16) for a in range(B)]
    xns = [np.ascontiguousarray(batch[a]).astype(bf16) for a in range(B)]
    lts = [np.ascontiguousarray(latent[a].T).astype(bf16) for a in range(B)]
    wkts, wqts, wvts, wus = [], [], [], []
    for hg in range(2):
        cols = slice(hg * NH, (hg + 1) * NH)
        wkts.append(np.ascontiguousarray(np.asarray(Wk, np.float32)[:, cols].T).astype(bf16))
        wqts.append(np.ascontiguousarray(np.asarray(Wq, np.float32)[:, cols].T).astype(bf16))
        wvts.append(np.ascontiguousarray(np.asarray(Wv, np.float32)[:, cols].T).astype(bf16))
        wus.append(np.ascontiguousarray(np.asarray(Wu, np.float32)[cols, :]).astype(bf16))

    in_maps = []
    for core in range(N_CORES):
        a, hg = core // 2, core % 2
        in_maps.append({
            "XT": xts[a], "XN": xns[a], "LT": lts[a],
            "WKT": wkts[hg], "WQT": wqts[hg], "WVT": wvts[hg], "WU": wus[hg],
        })

    _CACHE["in_maps"] = in_maps
    res = run_bass_kernel_spmd(nc, in_maps, core_ids=list(range(N_CORES)))

    out = np.empty((B, S, E), dtype=np.float32)
    for a in range(B):
        out[a] = res.results[2 * a]["O"] + res.results[2 * a + 1]["O"] + bu
    return out
